# revision 15
# baseline (speedup 1.0000x reference)
"""Per-entity linear head: out[n, e] = sum_h x[n, e, h] * W[e, h] + b[e].

Full inputs: cell_states (4, 512, 64, 1024) f32, W (64, 1024), b (64,).
Data-parallel over the flattened batch*seq dim across 8 cores (64 MiB of
x per core); W/b are tiny and replicated, host-duplicated to 128
partitions so no on-chip broadcast is ever needed.

Per core: x_core viewed as [16384, 1024] rows.  Reduce-tile tt puts row
128*tt + p on partition p, so partition p always owns entity
e = p % 64 and W needs only a [128, 1024] resident tile.  One fused DVE
scalar_tensor_tensor per tile computes y[:, tt] = sum_h(x * w) in a
single pass over the data (the elementwise product is discarded into a
stride-0 dummy); bias is a per-partition tensor_scalar_add.

Timing model (from the perfetto trace): end-to-end is
  preamble (~7.2us) + x stream (64 MiB at ~400 GB/s) + last-chunk
  completion receipt (~2us) + tail compute + y store + postamble.
A chunk's completion semaphore fires ~2us after its last byte lands,
and only then may its STTs run; DVE (1.26us/tile) barely keeps pace
with DMA (1.31us/tile), so whatever DVE lag exists at stream end is
pure added time.  Hence:
- the last 8 tiles are single-tile dma_starts into a dedicated
  zero-reuse pool (post-stream work = 1 STT, not 8; the dedicated pool
  is what makes this safe -- rotating-pool tail DMAs wait on
  buffer-free semaphores and starve the stream),
- chunk 0 is split 2+6 so the first completion sem fires ~6us earlier
  and DVE starts with minimal lag (a full 1,1,2,4 head taper instead
  starves the SP issue pipeline through buffer recycling -- measured
  224us),
- w/b ride one combined [128, 1025] tensor on the ACT (scalar) HWDGE
  ring: off the SP ring so the x stream starts first, but with HWDGE's
  prompt completion (the gpsimd SWDGE ring completes lazily, ~7us
  after issue),
- y is biased and stored in two pieces so the bulk store's completion
  receipt is off the critical path; only cols [120:128] remain at the
  end (4 KiB store).

Notes:
- bacc.Bacc + nc.compile() (not raw Bass): compile() splits multi-sem
  waits into EventSemaphore instructions (walrus here allows only one
  wait per instruction) and codegens InstISA subclasses.
- The fused DVE TENSOR_TENSOR_REDUCE (InstISA) compiles but faults at
  runtime on this terminal; InstTensorScalarPtr (scalar_tensor_tensor)
  with accum_out is the native-BIR equivalent and runs fine.
- bf16 STT was tried and is SLOWER (1466ns vs 1219ns: no 2x uop for
  STT, plus an ACT cast stage) -- keep f32.
- w lives in PSUM: the DVE reads it over its dedicated PSUM port,
  halving DVE's SBUF read traffic (which contends with the DMA write
  stream).  DMA can't target PSUM, so stage through SBUF and copy on
  the otherwise-idle ScalarE.
"""

import numpy as np

import concourse.bass as bass
import concourse.mybir as mybir
from concourse import bacc, bass_utils
from concourse.tile import TileContext

B, S, E, H = 4, 512, 64, 1024
N_CORES = 8
N = B * S                # 2048 flattened batch*seq rows
NPC = N // N_CORES       # 256 n-rows per core
R = NPC * E              # 16384 (n, e) rows of length H per core
P = 128                  # SBUF partitions
T = R // P               # 128 reduce tiles / output columns per core
G = 12                   # reduce tiles per main DMA (6 MiB each)
TAIL = 8                 # trailing single-tile DMAs (512 KiB each)
SPLIT = T - TAIL         # y cols [0:SPLIT] bias+store early
X_BUFS = 3


def build() -> bass.Bass:
    nc = bacc.Bacc("TRN2", target_bir_lowering=False, enable_asserts=False)
    x = nc.dram_tensor("x", [R, H], mybir.dt.float32, kind="ExternalInput")
    wb = nc.dram_tensor("wb", [P, H + 1], mybir.dt.float32, kind="ExternalInput")
    y = nc.dram_tensor("y", [P, T], mybir.dt.float32, kind="ExternalOutput")

    xt_rows = x.rearrange("(tt p) h -> tt p h", p=P)  # [T, P, H]

    with TileContext(nc) as tc:
        with (
            tc.tile_pool(name="xpool", bufs=X_BUFS) as xpool,
            tc.tile_pool(name="xtail", bufs=TAIL) as xtail,
            tc.tile_pool(name="consts", bufs=1) as consts,
            tc.tile_pool(name="wpsum", bufs=1, space="PSUM") as wpsum,
            # scratch (dummy product sink) stays in SBUF: putting it in
            # PSUM contends with the w reads on DVE's PSUM port
            tc.tile_pool(name="scratch", bufs=4) as scratch,
        ):
            wb_stage = consts.tile([P, H + 1], mybir.dt.float32)
            w_sb = wpsum.tile([P, H], mybir.dt.float32)
            y_sb = consts.tile([P, T], mybir.dt.float32)

            # w/b on the ACT HWDGE ring: prompt completion, SP ring free
            nc.scalar.dma_start(out=wb_stage[:], in_=wb[:])
            nc.scalar.copy(w_sb[:], wb_stage[:, 0:H])

            def stt(xtile, c):
                dummy = scratch.tile([P, 1], mybir.dt.float32)
                nc.vector.scalar_tensor_tensor(
                    out=dummy.broadcast_to((P, H)),
                    in0=xtile,
                    scalar=1.0,
                    in1=w_sb[:],
                    op0=mybir.AluOpType.mult,
                    op1=mybir.AluOpType.mult,
                    accum_out=y_sb[:, c : c + 1],
                )

            # main stream: growing head (earliest first completion sem
            # -> DVE starts ~12.5us), then G-tile chunks.  Bigger G
            # amortizes the ~0.7-1.3us per-dma_start completion
            # overhead on the single HWDGE queue (G=4 measured 336
            # GB/s, G=8 400 GB/s); X_BUFS=3 keeps buffer recycling
            # ~14us ahead of the descriptor-supply deadline.
            # at most 2 head allocations before the G-chunks: more than
            # X_BUFS-1 small head allocations gates chunk issue on STT
            # completions and starves the SP issue pipeline
            sizes = [2, 6] + [G] * 9 + [4]
            start = 0
            for n in sizes:
                xt = xpool.tile([P, n, H], mybir.dt.float32, tag="xt")
                nc.sync.dma_start(
                    out=xt[:],
                    in_=xt_rows[start : start + n].rearrange("t p h -> p t h"),
                )
                for i in range(n):
                    stt(xt[:, i], start + i)
                start += n

            # bulk bias + store overlaps the tail; ACT ring keeps the
            # store issue off the SP ring
            nc.vector.tensor_scalar_add(
                y_sb[:, 0:SPLIT], y_sb[:, 0:SPLIT], wb_stage[:, H : H + 1]
            )
            nc.scalar.dma_start(out=y[:, 0:SPLIT], in_=y_sb[:, 0:SPLIT])

            # tail: single-tile DMAs, zero buffer reuse
            for c in range(SPLIT, T):
                xt1 = xtail.tile([P, H], mybir.dt.float32, tag="xt1")
                nc.sync.dma_start(out=xt1[:], in_=xt_rows[c].rearrange("p h -> p h"))
                stt(xt1[:], c)

            nc.vector.tensor_scalar_add(
                y_sb[:, SPLIT:T], y_sb[:, SPLIT:T], wb_stage[:, H : H + 1]
            )
            nc.scalar.dma_start(out=y[:, SPLIT:T], in_=y_sb[:, SPLIT:T])
    nc.compile()
    return nc


def _prepare_in_maps(cell_states, W, b):
    x_all = np.ascontiguousarray(cell_states, dtype=np.float32).reshape(N * E, H)
    w2 = np.concatenate([W, W], axis=0)                  # [128, H]
    b2 = np.concatenate([b, b]).reshape(P, 1)            # [128, 1]
    wb2 = np.ascontiguousarray(
        np.concatenate([w2, b2], axis=1), dtype=np.float32
    )
    in_maps = []
    for c in range(N_CORES):
        xc = x_all[c * R : (c + 1) * R]
        in_maps.append({"x": xc, "wb": wb2})
    return in_maps


def _unshard(per_core_y):
    outs = []
    for y_raw in per_core_y:
        # y_raw[p, tt] = out[2*tt + p//64, p%64] within the core's 256 rows
        outs.append(
            np.asarray(y_raw).reshape(2, E, T).transpose(2, 0, 1).reshape(NPC, E)
        )
    return np.concatenate(outs, axis=0).reshape(B, S, E)


def kernel_with_results(trace=False, **inputs):
    nc = build()
    in_maps = _prepare_in_maps(inputs["cell_states"], inputs["W"], inputs["b"])
    res = bass_utils.run_bass_kernel_spmd(
        nc, in_maps, core_ids=list(range(N_CORES)), trace=trace
    )
    out = _unshard([r["y"] for r in res.results])
    return out, res


def kernel(**inputs) -> np.ndarray:
    out, _ = kernel_with_results(trace=False, **inputs)
    return out


# revision 16
# speedup vs baseline: 1.0185x; 1.0185x over previous
"""Per-entity linear head: out[n, e] = sum_h x[n, e, h] * W[e, h] + b[e].

Full inputs: cell_states (4, 512, 64, 1024) f32, W (64, 1024), b (64,).
Data-parallel over the flattened batch*seq dim across 8 cores (64 MiB of
x per core); W/b are tiny and replicated + host-rearranged so no
on-chip broadcast or cross-partition movement is ever needed.

Per core: x_core viewed as [16384, 1024] rows.  Paired (L=2) mapping:
the dma for a 2m-tile chunk at row base puts rows base + 256*m + 2q + j
(j=0,1) on partition q, so each DMA descriptor covers 2 consecutive
DRAM rows = 8 KiB contiguous (vs 4 KiB for the classic row-per-
partition mapping) -- half the descriptors per byte on the single
HWDGE queue.  Partition q always owns entities (2q)%64 and (2q+1)%64,
so W needs only a host-prepared [128, 2, 1024] resident tile (8 KiB/
partition, fits PSUM).  One fused DVE scalar_tensor_tensor per
(pair m, j) computes y[:, col] = sum_h(x * w_j); bias is two strided
per-partition tensor_scalar_adds.

Timing model (from the perfetto trace): end-to-end is
  preamble (~7us) + x stream + last-chunk completion receipt (~2us)
  + tail compute + y store + postamble.
A chunk's completion semaphore fires ~2us after its last byte lands,
and only then may its STTs run; DVE (1.26us/tile) barely keeps pace
with the stream (~1.31us/tile at 400 GB/s), so both engines must stay
saturated.  Structure rules measured the hard way:
- 4 MiB dma_starts are the sweet spot (G=4: 336 GB/s, G=8: 400 GB/s,
  G=12 with 4KB descriptors: ~340-360 GB/s -- a ~1024-descriptor ring
  limit makes bigger chunks stall the issue pipeline),
- at most 2 small head allocations ([2,6]) before the G-chunks; more
  gates chunk issue on STT completions and starves the SP pipeline,
- the last 8 tiles are 4 double-tile dma_starts into a dedicated
  zero-reuse pool (post-stream work = 2 STTs; a shared rotating pool
  makes tail DMAs wait on buffer-free semaphores),
- w/b ride one combined [128, 2050] tensor on the ACT (scalar) HWDGE
  ring: off the SP ring so the x stream starts first, with HWDGE's
  prompt completion (the gpsimd SWDGE ring completes ~7us after
  issue),
- y is biased and stored in two pieces so the bulk store's completion
  receipt is off the critical path.

Notes:
- bacc.Bacc + nc.compile() (not raw Bass): compile() splits multi-sem
  waits into EventSemaphore instructions (walrus here allows only one
  wait per instruction) and codegens InstISA subclasses.
- The fused DVE TENSOR_TENSOR_REDUCE (InstISA) compiles but faults at
  runtime on this terminal; InstTensorScalarPtr (scalar_tensor_tensor)
  with accum_out is the native-BIR equivalent and runs fine.
- bf16 STT is SLOWER (1466ns vs 1219ns: no 2x uop for STT) -- keep
  f32.  nc.gpsimd.scalar_tensor_tensor fails walrus codegen.
- w lives in PSUM: the DVE reads it over its dedicated PSUM port,
  halving DVE's SBUF read traffic (which contends with the DMA write
  stream).  DMA can't target PSUM, so stage through SBUF and copy on
  the otherwise-idle ScalarE.
"""

import numpy as np

import concourse.bass as bass
import concourse.mybir as mybir
from concourse import bacc, bass_utils
from concourse.tile import TileContext

B, S, E, H = 4, 512, 64, 1024
N_CORES = 8
N = B * S                # 2048 flattened batch*seq rows
NPC = N // N_CORES       # 256 n-rows per core
R = NPC * E              # 16384 (n, e) rows of length H per core
P = 128                  # SBUF partitions
T = R // P               # 128 reduce tiles / output columns per core
G = 8                    # reduce tiles per main DMA (4 MiB each)
TAIL = 8                 # trailing tiles: 4 double-tile DMAs (1 MiB)
SPLIT = T - TAIL         # y cols [0:SPLIT] bias+store early
X_BUFS = 4


def build() -> bass.Bass:
    nc = bacc.Bacc("TRN2", target_bir_lowering=False, enable_asserts=False)
    x = nc.dram_tensor("x", [R, H], mybir.dt.float32, kind="ExternalInput")
    wb = nc.dram_tensor("wb", [P, 2 * H + 2], mybir.dt.float32, kind="ExternalInput")
    y = nc.dram_tensor("y", [P, T], mybir.dt.float32, kind="ExternalOutput")

    def chunk_view(base_tile, n_tiles):
        # rows 128*base_tile + 256m + 2q + j -> partition q, free (m, j, h):
        # each (q, m) descriptor covers 2 consecutive rows = 8 KiB
        rbase = 128 * base_tile
        return x[rbase : rbase + 128 * n_tiles].rearrange(
            "(m q j) h -> q m j h", q=P, j=2
        )

    with TileContext(nc) as tc:
        with (
            tc.tile_pool(name="xpool", bufs=X_BUFS) as xpool,
            tc.tile_pool(name="xtail", bufs=TAIL // 2) as xtail,
            tc.tile_pool(name="consts", bufs=1) as consts,
            tc.tile_pool(name="wpsum", bufs=1, space="PSUM") as wpsum,
            # scratch (dummy product sink) stays in SBUF: putting it in
            # PSUM contends with the w reads on DVE's PSUM port
            tc.tile_pool(name="scratch", bufs=4) as scratch,
        ):
            wb_stage = consts.tile([P, 2 * H + 2], mybir.dt.float32)
            w_ps = wpsum.tile([P, 2, H], mybir.dt.float32)
            y_sb = consts.tile([P, T], mybir.dt.float32)

            # w/b on the ACT HWDGE ring: prompt completion, SP ring free
            nc.scalar.dma_start(out=wb_stage[:], in_=wb[:])
            nc.scalar.copy(
                w_ps[:], wb_stage[:, 0 : 2 * H].rearrange("p (j h) -> p j h", j=2)
            )

            def stt(xtile, j, c):
                dummy = scratch.tile([P, 1], mybir.dt.float32)
                nc.vector.scalar_tensor_tensor(
                    out=dummy.broadcast_to((P, H)),
                    in0=xtile,
                    scalar=1.0,
                    in1=w_ps[:, j],
                    op0=mybir.AluOpType.mult,
                    op1=mybir.AluOpType.mult,
                    accum_out=y_sb[:, c : c + 1],
                )

            def bias_store(c0, c1):
                yv = y_sb[:, c0:c1].rearrange("p (m j) -> p m j", j=2)
                for j in range(2):
                    nc.vector.tensor_scalar_add(
                        yv[:, :, j],
                        yv[:, :, j],
                        wb_stage[:, 2 * H + j : 2 * H + j + 1],
                    )
                nc.scalar.dma_start(out=y[:, c0:c1], in_=y_sb[:, c0:c1])

            # main stream: [2,6] head (early first completion sem ->
            # DVE starts ~13.6us without starving the issue pipeline),
            # then G-tile chunks
            sizes = [2, 6] + [G] * ((SPLIT - 8) // G)
            start = 0
            for n in sizes:
                xt = xpool.tile([P, n // 2, 2, H], mybir.dt.float32, tag="xt")
                nc.sync.dma_start(out=xt[:], in_=chunk_view(start, n))
                for m in range(n // 2):
                    for j in range(2):
                        stt(xt[:, m, j], j, start + 2 * m + j)
                start += n

            # bulk bias + store overlaps the tail; ACT ring keeps the
            # store issue off the SP ring
            bias_store(0, SPLIT)

            # tail: double-tile DMAs, zero buffer reuse
            for c in range(SPLIT, T, 2):
                xt1 = xtail.tile([P, 1, 2, H], mybir.dt.float32, tag="xt1")
                nc.sync.dma_start(out=xt1[:], in_=chunk_view(c, 2))
                for j in range(2):
                    stt(xt1[:, 0, j], j, c + j)

            bias_store(SPLIT, T)
    nc.compile()
    return nc


def _prepare_in_maps(cell_states, W, b):
    x_all = np.ascontiguousarray(cell_states, dtype=np.float32).reshape(N * E, H)
    q = np.arange(P)[:, None]                            # [128, 1]
    ent = (2 * q + np.arange(2)[None, :]) % E            # [128, 2]
    w_pair = np.asarray(W)[ent].reshape(P, 2 * H)        # [128, 2048]
    b_pair = np.asarray(b)[ent]                          # [128, 2]
    wb2 = np.ascontiguousarray(
        np.concatenate([w_pair, b_pair], axis=1), dtype=np.float32
    )
    in_maps = []
    for c in range(N_CORES):
        xc = x_all[c * R : (c + 1) * R]
        in_maps.append({"x": xc, "wb": wb2})
    return in_maps


def _unshard(per_core_y):
    outs = []
    for y_raw in per_core_y:
        # y_raw[q, 2*pidx + j] = out_row(256*pidx + 2q + j) within the
        # core's 16384 rows; row r -> (n = r//64, e = r%64)
        outs.append(
            np.asarray(y_raw)
            .reshape(P, T // 2, 2)
            .transpose(1, 0, 2)
            .reshape(NPC, E)
        )
    return np.concatenate(outs, axis=0).reshape(B, S, E)


def kernel_with_results(trace=False, **inputs):
    nc = build()
    in_maps = _prepare_in_maps(inputs["cell_states"], inputs["W"], inputs["b"])
    res = bass_utils.run_bass_kernel_spmd(
        nc, in_maps, core_ids=list(range(N_CORES)), trace=trace
    )
    out = _unshard([r["y"] for r in res.results])
    return out, res


def kernel(**inputs) -> np.ndarray:
    out, _ = kernel_with_results(trace=False, **inputs)
    return out
